# revision 1
# baseline (speedup 1.0000x reference)
"""GAT layer kernel for Trainium2 (Bass/Tile), 8-core data-parallel over batch.

Reference computation (B=16, N=1024, IN_DIM=128, H=4, D=64):
    h = (x @ W).reshape(B,N,H,D)
    e_src = einsum('bnhd,hd->bnh', h, a_src); e_dst likewise
    e[b,i,j,h] = leakyrelu(e_src[b,i,h] + e_dst[b,j,h], 0.2)
    alpha = softmax_j(where(adj[i,j], e, -inf))
    out[b,i,(h,d)] = sum_j alpha[b,i,j,h] * h[b,j,h,d]

Strategy (per core, 2 batches):
  - Scores are built TRANSPOSED: PT[j,i] = exp(lrelu(e_src[i]+e_dst[j])) * adjT[j,i]
    so the alpha@h contraction (over j) lands on TensorE partitions naturally.
  - Softmax max-subtraction is skipped (|e| <~ 10, exp is safe in fp32); the
    multiplicative 0/1 mask commutes with the normalization exactly.
  - e_src broadcast across partitions via a K=1 matmul with a ones row.
  - ACT does Lrelu(in + bias_col) and Exp; DVE does the mask multiply;
    PE accumulates out[i, d|rowsum] over j-chunks with a fused ones column.
"""

import os
import sys
from contextlib import ExitStack

import numpy as np

for _p in ("/opt/trn_rl_repo", "/root/.axon_site/_ro/trn_rl_repo"):
    if os.path.isdir(_p) and _p not in sys.path:
        sys.path.insert(0, _p)

import concourse.bass as bass
import concourse.mybir as mybir
import concourse.tile as tile

F32 = mybir.dt.float32
AF = mybir.ActivationFunctionType
ALU = mybir.AluOpType

B, N, IN_DIM, H, D = 16, 1024, 128, 4, 64
HD = H * D            # 256
NCORES = 8
BL = B // NCORES      # 2 batches per core
NEG_SLOPE = 0.2
NTC = N // 128        # 8 token/query/neighbor chunks


def _split_excess_waits(nc, max_waits=1):
    """Walrus codegen rejects compute instructions carrying more than one
    sync wait (single wait slot per ISA struct). Move the extras onto
    engine-matched NoOps inserted immediately before the instruction."""
    def _steal_nop(engine):
        engine.nop()
        for fn in nc.m.functions:
            for blk in fn.blocks:
                il = blk.instructions
                if il and type(il[-1]).__name__ == "InstNoOp":
                    nop = il[-1]
                    blk.instructions = il[:-1]
                    return nop
        raise RuntimeError("could not locate appended nop")

    for fn in nc.m.functions:
        for blk in fn.blocks:
            il = list(blk.instructions)
            out = []
            changed = False
            for inst in il:
                si = inst.sync_info
                if (type(inst).__name__ != "InstNoOp" and si is not None
                        and len(si.on_wait) > max_waits):
                    waits = list(si.on_wait)
                    for w in waits[max_waits:]:
                        nop = _steal_nop(nc.engines[inst.engine])
                        nop.sync_info = mybir.SyncInfo(on_wait=[w], on_update=[])
                        out.append(nop)
                    inst.sync_info = mybir.SyncInfo(
                        on_wait=waits[:max_waits], on_update=list(si.on_update))
                    changed = True
                out.append(inst)
            if changed:
                blk.instructions = out


def build_gat_program():
    nc = bass.Bass("TRN2", target_bir_lowering=False, debug=False)
    xT_d = nc.dram_tensor("xT", (BL, IN_DIM, N), F32, kind="ExternalInput").ap()
    W_d = nc.dram_tensor("W", (IN_DIM, HD), F32, kind="ExternalInput").ap()
    Waug_d = nc.dram_tensor("Waug", (IN_DIM, H * (D + 1)), F32, kind="ExternalInput").ap()
    Acat_d = nc.dram_tensor("Acat", (2 * IN_DIM, 2 * H), F32, kind="ExternalInput").ap()
    maskT_d = nc.dram_tensor("maskT", (N, N), F32, kind="ExternalInput").ap()
    out_d = nc.dram_tensor("out", (BL, N, HD), F32, kind="ExternalOutput").ap()

    with tile.TileContext(nc) as tc:
        with ExitStack() as ctx:
            _gat_body(ctx, tc, out_d, xT_d, W_d, Waug_d, Acat_d, maskT_d)
    _split_excess_waits(nc)
    return nc


def _gat_body(ctx, tc, out_d, xT_d, W_d, Waug_d, Acat_d, maskT_d):
    nc = tc.nc
    consts = ctx.enter_context(tc.tile_pool(name="consts", bufs=1))
    persist = ctx.enter_context(tc.tile_pool(name="persist", bufs=1))
    work = ctx.enter_context(tc.tile_pool(name="work", bufs=2))
    ptpool = ctx.enter_context(tc.tile_pool(name="ptpool", bufs=10))
    osb_pool = ctx.enter_context(tc.tile_pool(name="osb", bufs=2))
    ps_a = ctx.enter_context(tc.tile_pool(name="ps_a", bufs=2, space="PSUM"))
    ps_b = ctx.enter_context(tc.tile_pool(name="ps_b", bufs=4, space="PSUM"))

    # ---- constants / inputs resident in SBUF ----
    W_sb = consts.tile([128, HD], F32)
    nc.sync.dma_start(out=W_sb, in_=W_d)
    Waug_sb = consts.tile([128, H * (D + 1)], F32)
    nc.sync.dma_start(out=Waug_sb, in_=Waug_d)
    Acat_sb = consts.tile([128, 16], F32)  # [oc] chunks of (256,8) side by side
    for oc in range(2):
        nc.sync.dma_start(out=Acat_sb[:, oc * 8:(oc + 1) * 8],
                          in_=Acat_d[oc * 128:(oc + 1) * 128, :])
    ones1 = consts.tile([1, 128], F32)
    nc.vector.memset(ones1, 1.0)
    maskT_sb = consts.tile([128, NTC * N], F32)  # jc-chunk at cols jc*N
    for jc in range(NTC):
        nc.sync.dma_start(out=maskT_sb[:, jc * N:(jc + 1) * N],
                          in_=maskT_d[jc * 128:(jc + 1) * 128, :])
    xT_sb = consts.tile([128, BL * N], F32)
    for b in range(BL):
        nc.sync.dma_start(out=xT_sb[:, b * N:(b + 1) * N], in_=xT_d[b])

    # ---- per-batch persistent intermediates ----
    hT_sb = persist.tile([128, BL, 2, N], F32)         # [o-in-chunk, b, oc, t]
    haug_sb = persist.tile([128, BL, NTC * 260], F32)  # [j-in-chunk, b, jc*260 + h*65 + d]
    E_sb = persist.tile([128, BL, NTC * 8], F32)       # [t-in-chunk, b, tc*8 + a]
    E02_sb = persist.tile([128, BL, NTC * 8], F32)     # 0.2 * E_sb
    ET_sb = persist.tile([1, BL, H, N], F32)           # e_src rows on partition 0

    # ---- phase 1: h = x@W (two layouts), E = h@Acat, ET = Acat.T@hT ----
    for b in range(BL):
        for oc in range(2):
            hT_ps = ps_a.tile([128, N], F32, tag="ps_a")
            for n2 in range(2):
                nc.tensor.matmul(hT_ps[:, n2 * 512:(n2 + 1) * 512],
                                 lhsT=W_sb[:, oc * 128:(oc + 1) * 128],
                                 rhs=xT_sb[:, b * N + n2 * 512: b * N + (n2 + 1) * 512],
                                 start=True, stop=True)
            nc.vector.tensor_copy(hT_sb[:, b, oc, :], hT_ps)
        for tci in range(NTC):
            haug_ps = ps_b.tile([128, 260], F32, tag="ps_b")
            nc.tensor.matmul(haug_ps,
                             lhsT=xT_sb[:, b * N + tci * 128: b * N + (tci + 1) * 128],
                             rhs=Waug_sb, start=True, stop=True)
            nc.vector.tensor_copy(haug_sb[:, b, tci * 260:(tci + 1) * 260], haug_ps)
        nc.vector.memset(haug_sb[:, b, 64::65], 1.0)  # ones cols for rowsum
        for tci in range(NTC):
            E_ps = ps_b.tile([128, 8], F32, tag="ps_b")
            for oc in range(2):
                nc.tensor.matmul(E_ps,
                                 lhsT=hT_sb[:, b, oc, tci * 128:(tci + 1) * 128],
                                 rhs=Acat_sb[:, oc * 8:(oc + 1) * 8],
                                 start=(oc == 0), stop=(oc == 1))
            nc.vector.tensor_copy(E_sb[:, b, tci * 8:(tci + 1) * 8], E_ps)
        nc.vector.tensor_scalar(out=E02_sb[:, b, :], in0=E_sb[:, b, :],
                                scalar1=NEG_SLOPE, scalar2=None, op0=ALU.mult)
        for h in range(H):
            ET_ps = ps_a.tile([1, N], F32, tag="ps_a")
            for oc in range(2):
                for n2 in range(2):
                    nc.tensor.matmul(ET_ps[:, n2 * 512:(n2 + 1) * 512],
                                     lhsT=Acat_sb[:, oc * 8 + h: oc * 8 + h + 1],
                                     rhs=hT_sb[:, b, oc, n2 * 512:(n2 + 1) * 512],
                                     start=(oc == 0), stop=(oc == 1))
            nc.scalar.copy(ET_sb[0:1, b, h, :], ET_ps)

    # ---- phase 2: attention ----
    for b in range(BL):
        osb_t = osb_pool.tile([128, NTC * HD], F32)  # [i-in-chunk, ic*256 + h*64 + d]
        for h in range(H):
            ebc_ps = ps_a.tile([128, N], F32, tag="ps_a")  # e_src bcast to 128 parts
            for n2 in range(2):
                nc.tensor.matmul(ebc_ps[:, n2 * 512:(n2 + 1) * 512],
                                 lhsT=ones1,
                                 rhs=ET_sb[0:1, b, h, n2 * 512:(n2 + 1) * 512],
                                 start=True, stop=True)
            pts = []
            for jc in range(NTC):
                # exp(lrelu(y)) = max(exp(y), exp(0.2*y)); y = e_src[i] + e_dst[j]
                bias_col = E_sb[:, b, jc * 8 + 4 + h: jc * 8 + 4 + h + 1]
                bias02_col = E02_sb[:, b, jc * 8 + 4 + h: jc * 8 + 4 + h + 1]
                t1 = work.tile([128, N], F32, tag="t1")
                nc.scalar.activation(t1, ebc_ps, AF.Exp, bias=bias_col, scale=1.0)
                t2 = work.tile([128, N], F32, tag="t2")
                nc.scalar.activation(t2, ebc_ps, AF.Exp, bias=bias02_col,
                                     scale=NEG_SLOPE)
                mx = work.tile([128, N], F32, tag="mx")
                nc.vector.tensor_max(mx, t1, t2)
                pt = ptpool.tile([128, N], F32, tag="pt")
                nc.vector.tensor_mul(pt, mx, maskT_sb[:, jc * N:(jc + 1) * N])
                pts.append(pt)
            for ic in range(NTC):
                oacc_ps = ps_b.tile([128, 65], F32, tag="ps_b")
                for jc in range(NTC):
                    nc.tensor.matmul(oacc_ps,
                                     lhsT=pts[jc][:, ic * 128:(ic + 1) * 128],
                                     rhs=haug_sb[:, b, jc * 260 + h * 65: jc * 260 + (h + 1) * 65],
                                     start=(jc == 0), stop=(jc == NTC - 1))
                rcol = work.tile([128, 1], F32, tag="rcol")
                nc.vector.reciprocal(rcol, oacc_ps[:, 64:65])
                nc.vector.tensor_scalar(
                    out=osb_t[:, ic * HD + h * D: ic * HD + (h + 1) * D],
                    in0=oacc_ps[:, 0:64],
                    scalar1=rcol, scalar2=None, op0=ALU.mult)
        for ic in range(NTC):
            nc.sync.dma_start(out=out_d[b, ic * 128:(ic + 1) * 128, :],
                              in_=osb_t[:, ic * HD:(ic + 1) * HD])


def prep_inputs(x, adj, W, a_src, a_dst):
    """Host-side prep: shard x over cores, build augmented weight layouts."""
    x = np.asarray(x, np.float32)
    adj = np.asarray(adj)
    W = np.asarray(W, np.float32)
    a_src = np.asarray(a_src, np.float32)
    a_dst = np.asarray(a_dst, np.float32)

    maskT = np.ascontiguousarray(adj.T).astype(np.float32)
    Acat = np.zeros((2 * IN_DIM, 2 * H), np.float32)
    for h in range(H):
        Acat[h * D:(h + 1) * D, h] = a_src[h]
        Acat[h * D:(h + 1) * D, H + h] = a_dst[h]
    Waug = np.zeros((IN_DIM, H * (D + 1)), np.float32)
    for h in range(H):
        Waug[:, h * 65:h * 65 + 64] = W[:, h * 64:(h + 1) * 64]

    in_maps = []
    for c in range(NCORES):
        xT = np.ascontiguousarray(x[c * BL:(c + 1) * BL].transpose(0, 2, 1))
        in_maps.append({"xT": xT, "W": W, "Waug": Waug, "Acat": Acat,
                       "maskT": maskT})
    return in_maps


_PROGRAM_CACHE = {}


def _get_program():
    if "nc" not in _PROGRAM_CACHE:
        _PROGRAM_CACHE["nc"] = build_gat_program()
    return _PROGRAM_CACHE["nc"]


def run_on_hw(inputs, trace=False):
    from concourse.bass_utils import run_bass_kernel_spmd
    nc = _get_program()
    in_maps = prep_inputs(**inputs)
    res = run_bass_kernel_spmd(nc, in_maps, list(range(NCORES)), trace=trace)
    out = np.concatenate([res.results[c]["out"] for c in range(NCORES)], axis=0)
    return out, res


def kernel(**inputs) -> np.ndarray:
    out, _ = run_on_hw(inputs, trace=False)
    return out



# revision 26
# speedup vs baseline: 1.3340x; 1.3340x over previous
"""GAT layer kernel for Trainium2 (Bass/Tile), 8-core data-parallel over batch.

Reference computation (B=16, N=1024, IN_DIM=128, H=4, D=64):
    h = (x @ W).reshape(B,N,H,D)
    e_src = einsum('bnhd,hd->bnh', h, a_src); e_dst likewise
    e[b,i,j,h] = leakyrelu(e_src[b,i,h] + e_dst[b,j,h], 0.2)
    alpha = softmax_j(where(adj[i,j], e, -inf))
    out[b,i,(h,d)] = sum_j alpha[b,i,j,h] * h[b,j,h,d]

Strategy (per core, 2 batches x 4 heads = 8 attention units):
  - Scores are built TRANSPOSED, PT[j,i], so the alpha@h contraction (over j)
    lands on TensorE partitions naturally.
  - exp(lrelu(y)) = max(exp(y), exp(0.2 y)); a global shift of -7 (cancelled
    exactly by the softmax normalization) keeps everything in fp16 range.
  - Per (b,h) the 8 j-chunks are split between two pipelines to balance
    engines:
      E2 path:    ACT computes t1=Exp(ebc + (d-7)) and t2=Exp(0.2 ebc +
                  (0.2 d - 5.6)) straight from a PSUM s-broadcast with the
                  d-term fused as a per-partition bias column.
      OUTER path: exp factorizes over the score matrix: t1 = exp(d_j) (x)
                  exp(s_i - 7) is a rank-1 outer product computed by K=1
                  fp16 matmuls on the PE; likewise t2 with the 0.2-scaled
                  vectors.
    DVE then takes max(t1,t2) (2x fp16 mode on the E2 path) and applies the
    0/1 adjacency mask; a few masks per unit run on GPSIMD to offload DVE.
  - e_src/e_dst come from one small matmul x @ (W @ a) with host-prefused
    weights; row layouts are produced by PE transposes. Everything runs in
    fp16 (1 cycle/row on PE) with fp32 PSUM accumulation; softmax ratios are
    exact in fp32 (rowsum via a separate ones-column accumulation, one
    batched reciprocal per unit).
"""

import os
import sys
from contextlib import ExitStack

import numpy as np

for _p in ("/opt/trn_rl_repo", "/root/.axon_site/_ro/trn_rl_repo"):
    if os.path.isdir(_p) and _p not in sys.path:
        sys.path.insert(0, _p)

import concourse.bass as bass
import concourse.mybir as mybir
import concourse.tile as tile

F16 = mybir.dt.float16
F32 = mybir.dt.float32
AF = mybir.ActivationFunctionType
ALU = mybir.AluOpType

B, N, IN_DIM, H, D = 16, 1024, 128, 4, 64
HD = H * D            # 256
NCORES = 8
BL = B // NCORES      # 2 batches per core
NBH = BL * H          # 8 attention units per core
NEG_SLOPE = 0.2
NTC = N // 128        # 8 chunks of 128 nodes
SHIFT = 7.0           # global exp shift, cancels in softmax

KE = 5                # j-chunks 0..KE-1 via ACT (E2), KE..7 via PE outer
POOL_MASK_JC = (1, 5)      # j-chunks whose mask multiply runs on GPSIMD


def _split_excess_waits(nc, max_waits=1):
    """Walrus codegen rejects compute instructions carrying more than one
    sync wait (single wait slot per ISA struct). Move the extras onto
    engine-matched NoOps inserted immediately before the instruction."""
    def _steal_nop(engine):
        engine.nop()
        for fn in nc.m.functions:
            for blk in fn.blocks:
                il = blk.instructions
                if il and type(il[-1]).__name__ == "InstNoOp":
                    nop = il[-1]
                    blk.instructions = il[:-1]
                    return nop
        raise RuntimeError("could not locate appended nop")

    for fn in nc.m.functions:
        for blk in fn.blocks:
            il = list(blk.instructions)
            out = []
            changed = False
            for inst in il:
                si = inst.sync_info
                if (type(inst).__name__ != "InstNoOp" and si is not None
                        and len(si.on_wait) > max_waits):
                    waits = list(si.on_wait)
                    for w in waits[max_waits:]:
                        nop = _steal_nop(nc.engines[inst.engine])
                        nop.sync_info = mybir.SyncInfo(on_wait=[w], on_update=[])
                        out.append(nop)
                    inst.sync_info = mybir.SyncInfo(
                        on_wait=waits[:max_waits], on_update=list(si.on_update))
                    changed = True
                out.append(inst)
            if changed:
                blk.instructions = out


def build_gat_program():
    nc = bass.Bass("TRN2", target_bir_lowering=False, debug=False)
    xT_d = nc.dram_tensor("xT", (BL, IN_DIM, N), F16, kind="ExternalInput").ap()
    Wr_d = nc.dram_tensor("Wr", (IN_DIM, HD), F16, kind="ExternalInput").ap()
    wcat_d = nc.dram_tensor("wcat", (IN_DIM, 2 * H), F16, kind="ExternalInput").ap()
    ident_d = nc.dram_tensor("ident", (128, 128), F16, kind="ExternalInput").ap()
    maskT_d = nc.dram_tensor("maskT", (N, N), F16, kind="ExternalInput").ap()
    sel1_d = nc.dram_tensor("sel1", (4, H * 128), F16, kind="ExternalInput").ap()
    selc_d = nc.dram_tensor("selc", (4, H), F32, kind="ExternalInput").ap()
    out_d = nc.dram_tensor("out", (BL, N, HD), F16, kind="ExternalOutput").ap()

    with tile.TileContext(nc) as tc:
        with ExitStack() as ctx:
            _gat_body(ctx, tc, out_d, xT_d, Wr_d, wcat_d, ident_d, maskT_d,
                      sel1_d, selc_d)
    _split_excess_waits(nc)
    return nc


def _gat_body(ctx, tc, out_d, xT_d, Wr_d, wcat_d, ident_d, maskT_d,
              sel1_d, selc_d):
    nc = tc.nc
    consts = ctx.enter_context(tc.tile_pool(name="consts", bufs=1))
    persist = ctx.enter_context(tc.tile_pool(name="persist", bufs=1))
    t12 = ctx.enter_context(tc.tile_pool(name="t12", bufs=4))
    ptpool = ctx.enter_context(tc.tile_pool(name="ptpool", bufs=12))
    osb_pool = ctx.enter_context(tc.tile_pool(name="osb", bufs=2))
    work = ctx.enter_context(tc.tile_pool(name="work", bufs=2))
    # PSUM budget (8 banks): ebc 2 + ps_t 3 (shared with phase 1) + oacc 2
    # (all 8 i-chunk accumulators packed into one bank-sized tile) + rs 1.
    ps_ebc = ctx.enter_context(tc.tile_pool(name="ps_ebc", bufs=1, space="PSUM"))
    ps_t = ctx.enter_context(tc.tile_pool(name="ps_t", bufs=3, space="PSUM"))
    ps_o = ctx.enter_context(tc.tile_pool(name="ps_o", bufs=2, space="PSUM"))
    ps_rs = ctx.enter_context(tc.tile_pool(name="ps_rs", bufs=1, space="PSUM"))

    # ---- constants / inputs resident in SBUF ----
    Wr_sb = consts.tile([128, HD], F16)
    nc.sync.dma_start(out=Wr_sb, in_=Wr_d)
    wcat_sb = consts.tile([128, 2 * H], F16)
    nc.sync.dma_start(out=wcat_sb, in_=wcat_d)
    ident_sb = consts.tile([128, 128], F16)
    nc.sync.dma_start(out=ident_sb, in_=ident_d)
    m16 = consts.tile([128, NTC * N], F16)  # jc chunk at cols jc*N
    for jc in range(NTC):
        nc.sync.dma_start(out=m16[:, jc * N:(jc + 1) * N],
                          in_=maskT_d[jc * 128:(jc + 1) * 128, :])
    xT16 = consts.tile([128, BL * N], F16)
    for b in range(BL):
        nc.sync.dma_start(out=xT16[:, b * N:(b + 1) * N], in_=xT_d[b])
    ones1 = consts.tile([1, 128], F16)
    nc.vector.memset(ones1, 1.0)
    onescol = consts.tile([128, 1], F16)
    nc.vector.memset(onescol, 1.0)
    zero128 = consts.tile([128, 1], F32)
    nc.vector.memset(zero128, 0.0)
    m7col4 = consts.tile([4, 1], F32)
    nc.vector.memset(m7col4, -SHIFT)

    # ---- persistent intermediates ----
    E_sb = persist.tile([128, BL, NTC, H], F16)     # e_src columns
    E7_sb = persist.tile([128, BL, NTC, H], F16)    # d - 7 (E2 t1 bias)
    E02_sb = persist.tile([128, BL, NTC, H], F16)   # 0.2 d - 7 (E2 t2 bias)
    expdc2 = persist.tile([128, BL, NTC, H], F16)   # exp(0.2 d) columns
    srows4 = persist.tile([4, BL * N], F16)         # s rows [h, b*N+i]
    exps1 = persist.tile([4, BL * N], F16)          # exp(s - 7)
    exps2 = persist.tile([4, BL * N], F16)          # exp(0.2 s - 7)
    expd4_2 = persist.tile([4, BL * N], F16)        # exp(0.2 d) rows
    # per-head zeroed variants (other rows 0) so the whole tile is lhsT
    expz2 = [persist.tile([4, BL * N], F16, name=f"expz2_{h}")
             for h in range(H)]
    haug16 = persist.tile([128, BL, NTC, HD], F16)  # h in fp16, [j, b, jc, (h d)]
    # selector constants from host: sel row h = 1, others 0
    sel1_sb = consts.tile([4, H * 128], F16)
    nc.sync.dma_start(out=sel1_sb, in_=sel1_d)
    selc_sb = consts.tile([4, H], F32)
    nc.sync.dma_start(out=selc_sb, in_=selc_d)
    sel1 = [sel1_sb[:, h * 128:(h + 1) * 128] for h in range(H)]
    selc = [selc_sb[:, h:h + 1] for h in range(H)]

    # ---- phase 1: E = x @ wcat; exp vectors via PE transposes; h ----
    for b in range(BL):
        for tc_i in range(NTC):
            E_ps = ps_t.tile([128, 2 * H], F32, tag="ps_t")
            nc.tensor.matmul(E_ps,
                             lhsT=xT16[:, b * N + tc_i * 128:
                                       b * N + (tc_i + 1) * 128],
                             rhs=wcat_sb, start=True, stop=True)
            nc.vector.tensor_copy(E_sb[:, b, tc_i, :], E_ps[:, 0:4])
            nc.vector.tensor_scalar(out=E7_sb[:, b, tc_i, :],
                                    in0=E_ps[:, 4:8], scalar1=-SHIFT,
                                    scalar2=None, op0=ALU.add)
            nc.vector.tensor_scalar(out=E02_sb[:, b, tc_i, :],
                                    in0=E_ps[:, 4:8], scalar1=NEG_SLOPE,
                                    scalar2=-SHIFT,
                                    op0=ALU.mult, op1=ALU.add)
            nc.scalar.activation(expdc2[:, b, tc_i, :], E_ps[:, 4:8], AF.Exp,
                                 bias=zero128, scale=NEG_SLOPE)
    # s rows ([4, N] per b, base partition 0) -> exp on ACT
    for b in range(BL):
        srows_ps = ps_t.tile([4, N], F16, tag="ps_t")
        for tc_i in range(NTC):
            nc.tensor.transpose(srows_ps[:, tc_i * 128:(tc_i + 1) * 128],
                                in_=E_sb[:, b, tc_i, :], identity=ident_sb)
        nc.vector.tensor_copy(srows4[:, b * N:(b + 1) * N], srows_ps)
        nc.scalar.activation(exps1[:, b * N:(b + 1) * N], srows_ps, AF.Exp,
                             bias=m7col4, scale=1.0)
        nc.scalar.activation(exps2[:, b * N:(b + 1) * N], srows_ps, AF.Exp,
                             bias=m7col4, scale=NEG_SLOPE)
    # exp(0.2 d) rows: transpose the exp'd columns, then per-head zeroed rows
    for b in range(BL):
        pd_ps = ps_t.tile([4, N], F16, tag="ps_t", name=f"pd_{b}")
        for tc_i in range(NTC):
            nc.tensor.transpose(pd_ps[:, tc_i * 128:(tc_i + 1) * 128],
                                in_=expdc2[:, b, tc_i, :],
                                identity=ident_sb)
        nc.vector.tensor_copy(expd4_2[:, b * N:(b + 1) * N], pd_ps)
        for h in range(H):
            nc.gpsimd.tensor_scalar(
                out=expz2[h][:, b * N:(b + 1) * N],
                in0=expd4_2[:, b * N:(b + 1) * N],
                scalar1=selc[h], scalar2=None, op0=ALU.mult)
    # h in fp16 (per j-chunk layout for the contraction)
    for b in range(BL):
        for jc in range(NTC):
            h_ps = ps_t.tile([128, HD], F32, tag="ps_t")
            nc.tensor.matmul(h_ps,
                             lhsT=xT16[:, b * N + jc * 128:
                                       b * N + (jc + 1) * 128],
                             rhs=Wr_sb, start=True, stop=True)
            nc.vector.tensor_copy(haug16[:, b, jc, :], h_ps)

    # ---- phase 2: attention ----
    for bh in range(NBH):
        b, h = bh // H, bh % H
        ebc_ps = None
        if KE > 0:
            ebc_ps = ps_ebc.tile([128, N], F32, tag="ps_ebc")
            for n2 in range(2):
                nc.tensor.matmul(
                    ebc_ps[:, n2 * 512:(n2 + 1) * 512],
                    lhsT=sel1[h],
                    rhs=srows4[:, b * N + n2 * 512: b * N + (n2 + 1) * 512],
                    start=True, stop=True)
        pts = []
        for jc in range(NTC):
            pt = ptpool.tile([128, N], F16, tag="pt")
            if jc < KE:
                t1 = t12.tile([128, N], F16, tag="t1")
                nc.scalar.activation(t1, ebc_ps, AF.Exp,
                                     bias=E7_sb[:, b, jc, h:h + 1], scale=1.0)
                t2 = t12.tile([128, N], F16, tag="t2")
                nc.scalar.activation(t2, ebc_ps, AF.Exp,
                                     bias=E02_sb[:, b, jc, h:h + 1],
                                     scale=NEG_SLOPE)
                nc.vector.tensor_tensor(out=pt, in0=t1, in1=t2, op=ALU.max)
            else:
                t1 = t12.tile([128, N], F16, tag="t1")
                nc.scalar.activation(t1, ebc_ps, AF.Exp,
                                     bias=E7_sb[:, b, jc, h:h + 1], scale=1.0)
                for n2 in range(2):
                    tB = ps_t.tile([128, 512], F32, tag="ps_t")
                    nc.tensor.matmul(tB,
                                     lhsT=expz2[h][:, b * N + jc * 128:
                                                   b * N + (jc + 1) * 128],
                                     rhs=exps2[:, b * N + n2 * 512:
                                               b * N + (n2 + 1) * 512],
                                     start=True, stop=True)
                    nc.vector.tensor_tensor(out=pt[:, n2 * 512:(n2 + 1) * 512],
                                            in0=tB,
                                            in1=t1[:, n2 * 512:(n2 + 1) * 512],
                                            op=ALU.max)
            if jc in POOL_MASK_JC:
                nc.gpsimd.tensor_mul(pt, pt, m16[:, jc * N:(jc + 1) * N])
            else:
                nc.vector.tensor_mul(pt, pt, m16[:, jc * N:(jc + 1) * N])
            pts.append(pt)

        # rowsums first so the reciprocal overlaps the output matmuls
        rs_ps = ps_rs.tile([128, 8], F32, tag="ps_rs")
        for ic in range(NTC):
            for jc in range(NTC):
                nc.tensor.matmul(rs_ps[:, ic:ic + 1],
                                 lhsT=pts[jc][:, ic * 128:(ic + 1) * 128],
                                 rhs=onescol,
                                 start=(jc == 0), stop=(jc == NTC - 1))
        rcol = work.tile([128, 8], F32, tag="rcol")
        nc.vector.reciprocal(rcol, rs_ps)
        oacc = ps_o.tile([128, NTC, D], F32, tag="ps_o")
        if h == 0:
            osb_t = osb_pool.tile([128, NTC * HD], F16, tag="osb")
            _OSB[b] = osb_t
        osb_t = _OSB[b]
        for ic in range(NTC):
            for jc in range(NTC):
                nc.tensor.matmul(oacc[:, ic, :],
                                 lhsT=pts[jc][:, ic * 128:(ic + 1) * 128],
                                 rhs=haug16[:, b, jc, h * D:(h + 1) * D],
                                 start=(jc == 0), stop=(jc == NTC - 1))
            nc.scalar.mul(
                osb_t[:, ic * HD + h * D: ic * HD + (h + 1) * D],
                oacc[:, ic, :], rcol[:, ic:ic + 1])
        if h == H - 1:
            for ic in range(NTC):
                nc.sync.dma_start(out=out_d[b, ic * 128:(ic + 1) * 128, :],
                                  in_=osb_t[:, ic * HD:(ic + 1) * HD])


_OSB = {}


def prep_inputs(x, adj, W, a_src, a_dst):
    """Host-side prep: shard x over cores, fp16 layouts, fused a-weights."""
    x = np.asarray(x, np.float32)
    adj = np.asarray(adj)
    W = np.asarray(W, np.float32)
    a_src = np.asarray(a_src, np.float32)
    a_dst = np.asarray(a_dst, np.float32)

    maskT = np.ascontiguousarray(adj.T).astype(np.float16)
    wcat = np.zeros((IN_DIM, 2 * H), np.float32)
    for h in range(H):
        wcat[:, h] = W[:, h * D:(h + 1) * D] @ a_src[h]
        wcat[:, H + h] = W[:, h * D:(h + 1) * D] @ a_dst[h]
    wcat16 = wcat.astype(np.float16)
    Wr16 = W.astype(np.float16)
    ident16 = np.eye(128, dtype=np.float16)

    sel1 = np.zeros((4, H * 128), np.float16)
    selc = np.zeros((4, H), np.float32)
    for h in range(H):
        sel1[h, h * 128:(h + 1) * 128] = 1.0
        selc[h, h] = 1.0

    in_maps = []
    for c in range(NCORES):
        xT = np.ascontiguousarray(
            x[c * BL:(c + 1) * BL].transpose(0, 2, 1)).astype(np.float16)
        in_maps.append({"xT": xT, "Wr": Wr16, "wcat": wcat16,
                        "ident": ident16, "maskT": maskT,
                        "sel1": sel1, "selc": selc})
    return in_maps


_PROGRAM_CACHE = {}


def _get_program():
    if "nc" not in _PROGRAM_CACHE:
        _OSB.clear()
        _PROGRAM_CACHE["nc"] = build_gat_program()
    return _PROGRAM_CACHE["nc"]


def run_on_hw(inputs, trace=False):
    from concourse.bass_utils import run_bass_kernel_spmd
    nc = _get_program()
    in_maps = prep_inputs(**inputs)
    res = run_bass_kernel_spmd(nc, in_maps, list(range(NCORES)), trace=trace)
    out = np.concatenate(
        [np.asarray(res.results[c]["out"], np.float32) for c in range(NCORES)],
        axis=0)
    return out, res


def kernel(**inputs) -> np.ndarray:
    out, _ = run_on_hw(inputs, trace=False)
    return out


# revision 27
# speedup vs baseline: 1.4504x; 1.0872x over previous
"""GAT layer kernel for Trainium2 (Bass/Tile), 8-core data-parallel over batch.

Reference computation (B=16, N=1024, IN_DIM=128, H=4, D=64):
    h = (x @ W).reshape(B,N,H,D)
    e_src = einsum('bnhd,hd->bnh', h, a_src); e_dst likewise
    e[b,i,j,h] = leakyrelu(e_src[b,i,h] + e_dst[b,j,h], 0.2)
    alpha = softmax_j(where(adj[i,j], e, -inf))
    out[b,i,(h,d)] = sum_j alpha[b,i,j,h] * h[b,j,h,d]

Strategy (per core, 2 batches x 4 heads = 8 attention units):
  - Scores are built TRANSPOSED, PT[j,i], so the alpha@h contraction (over j)
    lands on TensorE partitions naturally.
  - exp(lrelu(y)) = max(exp(y), exp(0.2 y)); a global shift of -7 (cancelled
    exactly by the softmax normalization) keeps everything in fp16 range.
  - Per (b,h) the 8 j-chunks are split between two pipelines to balance
    engines:
      E2 path:    ACT computes t1=Exp(ebc + (d-7)) and t2=Exp(0.2 ebc +
                  (0.2 d - 5.6)) straight from a PSUM s-broadcast with the
                  d-term fused as a per-partition bias column.
      OUTER path: exp factorizes over the score matrix: t1 = exp(d_j) (x)
                  exp(s_i - 7) is a rank-1 outer product computed by K=1
                  fp16 matmuls on the PE; likewise t2 with the 0.2-scaled
                  vectors.
    DVE then takes max(t1,t2) (2x fp16 mode on the E2 path) and applies the
    0/1 adjacency mask; a few masks per unit run on GPSIMD to offload DVE.
  - e_src/e_dst come from one small matmul x @ (W @ a) with host-prefused
    weights; row layouts are produced by PE transposes. Everything runs in
    fp16 (1 cycle/row on PE) with fp32 PSUM accumulation; softmax ratios are
    exact in fp32 (rowsum via a separate ones-column accumulation, one
    batched reciprocal per unit).
"""

import os
import sys
from contextlib import ExitStack

import numpy as np

for _p in ("/opt/trn_rl_repo", "/root/.axon_site/_ro/trn_rl_repo"):
    if os.path.isdir(_p) and _p not in sys.path:
        sys.path.insert(0, _p)

import concourse.bass as bass
import concourse.mybir as mybir
import concourse.tile as tile

F16 = mybir.dt.float16
F32 = mybir.dt.float32
AF = mybir.ActivationFunctionType
ALU = mybir.AluOpType

B, N, IN_DIM, H, D = 16, 1024, 128, 4, 64
HD = H * D            # 256
NCORES = 8
BL = B // NCORES      # 2 batches per core
NBH = BL * H          # 8 attention units per core
NEG_SLOPE = 0.2
NTC = N // 128        # 8 chunks of 128 nodes
SHIFT = 7.0           # global exp shift, cancels in softmax

KE = 3                # j-chunks 0..KE-1 via ACT (E2), KE..7 via PE outer
POOL_MASK_JC = (1, 4, 6)   # j-chunks whose mask multiply runs on GPSIMD


def _split_excess_waits(nc, max_waits=1):
    """Walrus codegen rejects compute instructions carrying more than one
    sync wait (single wait slot per ISA struct). Move the extras onto
    engine-matched NoOps inserted immediately before the instruction."""
    def _steal_nop(engine):
        engine.nop()
        for fn in nc.m.functions:
            for blk in fn.blocks:
                il = blk.instructions
                if il and type(il[-1]).__name__ == "InstNoOp":
                    nop = il[-1]
                    blk.instructions = il[:-1]
                    return nop
        raise RuntimeError("could not locate appended nop")

    for fn in nc.m.functions:
        for blk in fn.blocks:
            il = list(blk.instructions)
            out = []
            changed = False
            for inst in il:
                si = inst.sync_info
                if (type(inst).__name__ != "InstNoOp" and si is not None
                        and len(si.on_wait) > max_waits):
                    waits = list(si.on_wait)
                    for w in waits[max_waits:]:
                        nop = _steal_nop(nc.engines[inst.engine])
                        nop.sync_info = mybir.SyncInfo(on_wait=[w], on_update=[])
                        out.append(nop)
                    inst.sync_info = mybir.SyncInfo(
                        on_wait=waits[:max_waits], on_update=list(si.on_update))
                    changed = True
                out.append(inst)
            if changed:
                blk.instructions = out


def build_gat_program():
    nc = bass.Bass("TRN2", target_bir_lowering=False, debug=False)
    xT_d = nc.dram_tensor("xT", (BL, IN_DIM, N), F16, kind="ExternalInput").ap()
    Wr_d = nc.dram_tensor("Wr", (IN_DIM, HD), F16, kind="ExternalInput").ap()
    wcat_d = nc.dram_tensor("wcat", (IN_DIM, 2 * H), F16, kind="ExternalInput").ap()
    ident_d = nc.dram_tensor("ident", (128, 128), F16, kind="ExternalInput").ap()
    maskT_d = nc.dram_tensor("maskT", (N, N), F16, kind="ExternalInput").ap()
    sel1_d = nc.dram_tensor("sel1", (4, H * 128), F16, kind="ExternalInput").ap()
    selc_d = nc.dram_tensor("selc", (4, H), F32, kind="ExternalInput").ap()
    out_d = nc.dram_tensor("out", (BL, N, HD), F16, kind="ExternalOutput").ap()

    with tile.TileContext(nc) as tc:
        with ExitStack() as ctx:
            _gat_body(ctx, tc, out_d, xT_d, Wr_d, wcat_d, ident_d, maskT_d,
                      sel1_d, selc_d)
    _split_excess_waits(nc)
    return nc


def _gat_body(ctx, tc, out_d, xT_d, Wr_d, wcat_d, ident_d, maskT_d,
              sel1_d, selc_d):
    nc = tc.nc
    consts = ctx.enter_context(tc.tile_pool(name="consts", bufs=1))
    persist = ctx.enter_context(tc.tile_pool(name="persist", bufs=1))
    t12 = ctx.enter_context(tc.tile_pool(name="t12", bufs=4))
    ptpool = ctx.enter_context(tc.tile_pool(name="ptpool", bufs=12))
    osb_pool = ctx.enter_context(tc.tile_pool(name="osb", bufs=2))
    work = ctx.enter_context(tc.tile_pool(name="work", bufs=2))
    # PSUM budget (8 banks): ebc 2 + ps_t 3 (shared with phase 1) + oacc 2
    # (all 8 i-chunk accumulators packed into one bank-sized tile) + rs 1.
    ps_ebc = ctx.enter_context(tc.tile_pool(name="ps_ebc", bufs=1, space="PSUM"))
    ps_t = ctx.enter_context(tc.tile_pool(name="ps_t", bufs=3, space="PSUM"))
    ps_o = ctx.enter_context(tc.tile_pool(name="ps_o", bufs=2, space="PSUM"))
    ps_rs = ctx.enter_context(tc.tile_pool(name="ps_rs", bufs=1, space="PSUM"))

    # ---- constants / inputs resident in SBUF ----
    Wr_sb = consts.tile([128, HD], F16)
    nc.sync.dma_start(out=Wr_sb, in_=Wr_d)
    wcat_sb = consts.tile([128, 2 * H], F16)
    nc.sync.dma_start(out=wcat_sb, in_=wcat_d)
    ident_sb = consts.tile([128, 128], F16)
    nc.sync.dma_start(out=ident_sb, in_=ident_d)
    m16 = consts.tile([128, NTC * N], F16)  # jc chunk at cols jc*N
    for jc in range(NTC):
        nc.sync.dma_start(out=m16[:, jc * N:(jc + 1) * N],
                          in_=maskT_d[jc * 128:(jc + 1) * 128, :])
    xT16 = consts.tile([128, BL * N], F16)
    for b in range(BL):
        nc.sync.dma_start(out=xT16[:, b * N:(b + 1) * N], in_=xT_d[b])
    ones1 = consts.tile([1, 128], F16)
    nc.vector.memset(ones1, 1.0)
    onescol = consts.tile([128, 1], F16)
    nc.vector.memset(onescol, 1.0)
    zero128 = consts.tile([128, 1], F32)
    nc.vector.memset(zero128, 0.0)
    m7col4 = consts.tile([4, 1], F32)
    nc.vector.memset(m7col4, -SHIFT)

    # ---- persistent intermediates ----
    E_sb = persist.tile([128, BL, NTC, H], F16)     # e_src columns
    E7_sb = persist.tile([128, BL, NTC, H], F16)    # d - 7 (E2 t1 bias)
    E02_sb = persist.tile([128, BL, NTC, H], F16)   # 0.2 d - 7 (E2 t2 bias)
    expdc2 = persist.tile([128, BL, NTC, H], F16)   # exp(0.2 d) columns
    srows4 = persist.tile([4, BL * N], F16)         # s rows [h, b*N+i]
    exps1 = persist.tile([4, BL * N], F16)          # exp(s - 7)
    exps2 = persist.tile([4, BL * N], F16)          # exp(0.2 s - 7)
    expd4_2 = persist.tile([4, BL * N], F16)        # exp(0.2 d) rows
    # per-head zeroed variants (other rows 0) so the whole tile is lhsT
    expz2 = [persist.tile([4, BL * N], F16, name=f"expz2_{h}")
             for h in range(H)]
    haug16 = persist.tile([128, BL, NTC, HD], F16)  # h in fp16, [j, b, jc, (h d)]
    # selector constants from host: sel row h = 1, others 0
    sel1_sb = consts.tile([4, H * 128], F16)
    nc.sync.dma_start(out=sel1_sb, in_=sel1_d)
    selc_sb = consts.tile([4, H], F32)
    nc.sync.dma_start(out=selc_sb, in_=selc_d)
    sel1 = [sel1_sb[:, h * 128:(h + 1) * 128] for h in range(H)]
    selc = [selc_sb[:, h:h + 1] for h in range(H)]

    # ---- phase 1: E = x @ wcat; exp vectors via PE transposes; h ----
    for b in range(BL):
        for tc_i in range(NTC):
            E_ps = ps_t.tile([128, 2 * H], F32, tag="ps_t")
            nc.tensor.matmul(E_ps,
                             lhsT=xT16[:, b * N + tc_i * 128:
                                       b * N + (tc_i + 1) * 128],
                             rhs=wcat_sb, start=True, stop=True)
            nc.vector.tensor_copy(E_sb[:, b, tc_i, :], E_ps[:, 0:4])
            nc.vector.tensor_scalar(out=E7_sb[:, b, tc_i, :],
                                    in0=E_ps[:, 4:8], scalar1=-SHIFT,
                                    scalar2=None, op0=ALU.add)
            nc.vector.tensor_scalar(out=E02_sb[:, b, tc_i, :],
                                    in0=E_ps[:, 4:8], scalar1=NEG_SLOPE,
                                    scalar2=-SHIFT,
                                    op0=ALU.mult, op1=ALU.add)
            nc.scalar.activation(expdc2[:, b, tc_i, :], E_ps[:, 4:8], AF.Exp,
                                 bias=zero128, scale=NEG_SLOPE)
    # s rows ([4, N] per b, base partition 0) -> exp on ACT
    for b in range(BL):
        srows_ps = ps_t.tile([4, N], F16, tag="ps_t")
        for tc_i in range(NTC):
            nc.tensor.transpose(srows_ps[:, tc_i * 128:(tc_i + 1) * 128],
                                in_=E_sb[:, b, tc_i, :], identity=ident_sb)
        nc.vector.tensor_copy(srows4[:, b * N:(b + 1) * N], srows_ps)
        nc.scalar.activation(exps1[:, b * N:(b + 1) * N], srows_ps, AF.Exp,
                             bias=m7col4, scale=1.0)
        nc.scalar.activation(exps2[:, b * N:(b + 1) * N], srows_ps, AF.Exp,
                             bias=m7col4, scale=NEG_SLOPE)
    # exp(0.2 d) rows: transpose the exp'd columns, then per-head zeroed rows
    for b in range(BL):
        pd_ps = ps_t.tile([4, N], F16, tag="ps_t", name=f"pd_{b}")
        for tc_i in range(NTC):
            nc.tensor.transpose(pd_ps[:, tc_i * 128:(tc_i + 1) * 128],
                                in_=expdc2[:, b, tc_i, :],
                                identity=ident_sb)
        nc.vector.tensor_copy(expd4_2[:, b * N:(b + 1) * N], pd_ps)
        for h in range(H):
            nc.gpsimd.tensor_scalar(
                out=expz2[h][:, b * N:(b + 1) * N],
                in0=expd4_2[:, b * N:(b + 1) * N],
                scalar1=selc[h], scalar2=None, op0=ALU.mult)
    # h in fp16 (per j-chunk layout for the contraction)
    for b in range(BL):
        for jc in range(NTC):
            h_ps = ps_t.tile([128, HD], F32, tag="ps_t")
            nc.tensor.matmul(h_ps,
                             lhsT=xT16[:, b * N + jc * 128:
                                       b * N + (jc + 1) * 128],
                             rhs=Wr_sb, start=True, stop=True)
            nc.vector.tensor_copy(haug16[:, b, jc, :], h_ps)

    # ---- phase 2: attention ----
    for bh in range(NBH):
        b, h = bh // H, bh % H
        ebc_ps = None
        if KE > 0:
            ebc_ps = ps_ebc.tile([128, N], F32, tag="ps_ebc")
            for n2 in range(2):
                nc.tensor.matmul(
                    ebc_ps[:, n2 * 512:(n2 + 1) * 512],
                    lhsT=sel1[h],
                    rhs=srows4[:, b * N + n2 * 512: b * N + (n2 + 1) * 512],
                    start=True, stop=True)
        pts = []
        for jc in range(NTC):
            pt = ptpool.tile([128, N], F16, tag="pt")
            if jc < KE:
                t1 = t12.tile([128, N], F16, tag="t1")
                nc.scalar.activation(t1, ebc_ps, AF.Exp,
                                     bias=E7_sb[:, b, jc, h:h + 1], scale=1.0)
                t2 = t12.tile([128, N], F16, tag="t2")
                nc.scalar.activation(t2, ebc_ps, AF.Exp,
                                     bias=E02_sb[:, b, jc, h:h + 1],
                                     scale=NEG_SLOPE)
                nc.vector.tensor_tensor(out=pt, in0=t1, in1=t2, op=ALU.max)
            else:
                t1 = t12.tile([128, N], F16, tag="t1")
                nc.scalar.activation(t1, ebc_ps, AF.Exp,
                                     bias=E7_sb[:, b, jc, h:h + 1], scale=1.0)
                for n2 in range(2):
                    tB = ps_t.tile([128, 512], F32, tag="ps_t")
                    nc.tensor.matmul(tB,
                                     lhsT=expz2[h][:, b * N + jc * 128:
                                                   b * N + (jc + 1) * 128],
                                     rhs=exps2[:, b * N + n2 * 512:
                                               b * N + (n2 + 1) * 512],
                                     start=True, stop=True)
                    nc.vector.tensor_tensor(out=pt[:, n2 * 512:(n2 + 1) * 512],
                                            in0=tB,
                                            in1=t1[:, n2 * 512:(n2 + 1) * 512],
                                            op=ALU.max)
            if jc in POOL_MASK_JC:
                nc.gpsimd.tensor_mul(pt, pt, m16[:, jc * N:(jc + 1) * N])
            else:
                nc.vector.tensor_mul(pt, pt, m16[:, jc * N:(jc + 1) * N])
            pts.append(pt)

        # rowsums first so the reciprocal overlaps the output matmuls
        rs_ps = ps_rs.tile([128, 8], F32, tag="ps_rs")
        for ic in range(NTC):
            for jc in range(NTC):
                nc.tensor.matmul(rs_ps[:, ic:ic + 1],
                                 lhsT=pts[jc][:, ic * 128:(ic + 1) * 128],
                                 rhs=onescol,
                                 start=(jc == 0), stop=(jc == NTC - 1))
        rcol = work.tile([128, 8], F32, tag="rcol")
        nc.vector.reciprocal(rcol, rs_ps)
        oacc = ps_o.tile([128, NTC, D], F32, tag="ps_o")
        if h == 0:
            osb_t = osb_pool.tile([128, NTC * HD], F16, tag="osb")
            _OSB[b] = osb_t
        osb_t = _OSB[b]
        for ic in range(NTC):
            for jc in range(NTC):
                nc.tensor.matmul(oacc[:, ic, :],
                                 lhsT=pts[jc][:, ic * 128:(ic + 1) * 128],
                                 rhs=haug16[:, b, jc, h * D:(h + 1) * D],
                                 start=(jc == 0), stop=(jc == NTC - 1))
            if ic % 2 == 0:
                nc.scalar.mul(
                    osb_t[:, ic * HD + h * D: ic * HD + (h + 1) * D],
                    oacc[:, ic, :], rcol[:, ic:ic + 1])
            else:
                nc.vector.tensor_scalar(
                    out=osb_t[:, ic * HD + h * D: ic * HD + (h + 1) * D],
                    in0=oacc[:, ic, :], scalar1=rcol[:, ic:ic + 1],
                    scalar2=None, op0=ALU.mult)
        if h == H - 1:
            for ic in range(NTC):
                nc.sync.dma_start(out=out_d[b, ic * 128:(ic + 1) * 128, :],
                                  in_=osb_t[:, ic * HD:(ic + 1) * HD])


_OSB = {}


def prep_inputs(x, adj, W, a_src, a_dst):
    """Host-side prep: shard x over cores, fp16 layouts, fused a-weights."""
    x = np.asarray(x, np.float32)
    adj = np.asarray(adj)
    W = np.asarray(W, np.float32)
    a_src = np.asarray(a_src, np.float32)
    a_dst = np.asarray(a_dst, np.float32)

    maskT = np.ascontiguousarray(adj.T).astype(np.float16)
    wcat = np.zeros((IN_DIM, 2 * H), np.float32)
    for h in range(H):
        wcat[:, h] = W[:, h * D:(h + 1) * D] @ a_src[h]
        wcat[:, H + h] = W[:, h * D:(h + 1) * D] @ a_dst[h]
    wcat16 = wcat.astype(np.float16)
    Wr16 = W.astype(np.float16)
    ident16 = np.eye(128, dtype=np.float16)

    sel1 = np.zeros((4, H * 128), np.float16)
    selc = np.zeros((4, H), np.float32)
    for h in range(H):
        sel1[h, h * 128:(h + 1) * 128] = 1.0
        selc[h, h] = 1.0

    in_maps = []
    for c in range(NCORES):
        xT = np.ascontiguousarray(
            x[c * BL:(c + 1) * BL].transpose(0, 2, 1)).astype(np.float16)
        in_maps.append({"xT": xT, "Wr": Wr16, "wcat": wcat16,
                        "ident": ident16, "maskT": maskT,
                        "sel1": sel1, "selc": selc})
    return in_maps


_PROGRAM_CACHE = {}


def _get_program():
    if "nc" not in _PROGRAM_CACHE:
        _OSB.clear()
        _PROGRAM_CACHE["nc"] = build_gat_program()
    return _PROGRAM_CACHE["nc"]


def run_on_hw(inputs, trace=False):
    from concourse.bass_utils import run_bass_kernel_spmd
    nc = _get_program()
    in_maps = prep_inputs(**inputs)
    res = run_bass_kernel_spmd(nc, in_maps, list(range(NCORES)), trace=trace)
    out = np.concatenate(
        [np.asarray(res.results[c]["out"], np.float32) for c in range(NCORES)],
        axis=0)
    return out, res


def kernel(**inputs) -> np.ndarray:
    out, _ = run_on_hw(inputs, trace=False)
    return out
